# revision 54
# baseline (speedup 1.0000x reference)
"""GQA attention (B=2,T=2048,D=2048,H=16,KV=4,HD=128, causal+RoPE) on 8 trn2 cores.

Sharding: 4-way head tensor-parallel x 2-way batch data-parallel.
Core c: batch b=c//4, TP shard s=c%4 -> q heads [4s..4s+3], kv head s.

Final design (~265us, vs 446us baseline):
- Host folds the RoPE even/odd gather and the 1/sqrt(HD) score scale into
  Wq/Wk, pre-blocks all weights into [128, j, cols] DMA-friendly layout,
  and sums the 4 TP partial outputs per batch.
- K and V^T projections run j-outer, interleaved with the 16-block xT DMA
  stream; V^T -> V (PV lhsT layout) via one XBAR dma_start_transpose.
- Scores are computed TRANSPOSED on the PE (ST[k,q] = kT_blk^T @ qT chunk,
  512 q wide, diagonal blocks column-trimmed), so exp writes P^T straight
  from PSUM to SBUF -- no PE/DMA transposes per block, no tensor_scalar
  (which stalls badly under XBAR traffic).
- Softmax denominator: an ones-weights matmul accumulated over P^T blocks
  lands l broadcast across all partitions; reciprocal_approx_fast (DVE
  reciprocal is ~8ns/elem; approx is 5x faster) then one tensor_mul
  normalizes O^T = V^T @ P^T straight out of PSUM.
- Causal mask applied by an accumulating rank-structured matmul
  (Ltri^T @ Rneg = -1e9*[q<k]) instead of a DVE add; exp batched over pairs
  of full score blocks ([128,1024] 2-bank PSUM tiles).
- Wo consumes O^T per 128-token block (paired 2-bank psums, one copy each);
  output copies on DVE, stores on the (transpose-free) SP queue.
- Q psums round-robin across all three PSUM rings (6-deep effective) so the
  serial DVE rope chain never gates the projection; OT of the last head is
  deferred past the next chunk's first S-phase for exp slack.
- PSUM: 2x[128,1024] score/proj ring + 2+2 [128,512] rings for L/O^T
  accumulators (one accumulation group per 2KB bank -- interleaved groups
  within a bank silently corrupt).
"""

import math
import os
import numpy as np

try:
    import concourse.bass as bass
except ImportError:  # pragma: no cover
    import sys

    sys.path.insert(0, "/opt/trn_rl_repo")
    import concourse.bass as bass

import concourse.mybir as mybir
import concourse.bacc as bacc
from concourse import bass_utils
from concourse.tile import TileContext
from contextlib import ExitStack
from ml_dtypes import bfloat16

B, T, D = 2, 2048, 2048
H, KV, HD = 16, 4, 128
TP = 4  # head-TP ways
NH = H // TP  # q heads per core = 4
NKB = D // 128  # 16 contraction blocks
NTC = T // 512  # 4 free-dim chunks
NTB = T // 128  # 16 token blocks
SCALE = 1.0 / math.sqrt(HD)
F32 = mybir.dt.float32
BF16 = mybir.dt.bfloat16
EXP = mybir.ActivationFunctionType.Exp
MASK_VAL = -1e9

_program = None
_last_results = None
last_exec_time_ns = None


def _build_program():
    global _program
    if _program is not None:
        return _program

    nc = bacc.Bacc(
        "TRN2",
        target_bir_lowering=False,
        debug=False,
        enable_asserts=False,
        num_devices=8,
    )
    xT_d = nc.dram_tensor("xT", [D, T], BF16, kind="ExternalInput").ap()
    wq_d = nc.dram_tensor("Wq", [128, NKB * NH * HD], BF16, kind="ExternalInput").ap()
    wk_d = nc.dram_tensor("Wk", [128, NKB * HD], BF16, kind="ExternalInput").ap()
    wv_d = nc.dram_tensor("Wv", [128, NKB * HD], BF16, kind="ExternalInput").ap()
    wo_d = nc.dram_tensor("Wo", [128, NH * D], BF16, kind="ExternalInput").ap()
    cos_d = nc.dram_tensor("cos", [128, T], BF16, kind="ExternalInput").ap()
    sin_d = nc.dram_tensor("sin", [128, T], BF16, kind="ExternalInput").ap()
    ltri_d = nc.dram_tensor("Ltri", [128, 128], BF16, kind="ExternalInput").ap()
    rneg_d = nc.dram_tensor("Rneg", [128, 128], BF16, kind="ExternalInput").ap()

    y_d = nc.dram_tensor("y", [T, D], BF16, kind="ExternalOutput").ap()

    with TileContext(nc) as tc, ExitStack() as ctx:
        big = ctx.enter_context(tc.tile_pool(name="big", bufs=1))
        psA = ctx.enter_context(tc.tile_pool(name="psA", bufs=2, space="PSUM"))
        psV = ctx.enter_context(tc.tile_pool(name="psV", bufs=2, space="PSUM"))
        psB = ctx.enter_context(tc.tile_pool(name="psB", bufs=2, space="PSUM"))
        rtmp = ctx.enter_context(tc.tile_pool(name="rtmp", bufs=1))
        ptpool = ctx.enter_context(tc.tile_pool(name="ptpool", bufs=2))
        opool = ctx.enter_context(tc.tile_pool(name="opool", bufs=3))
        ypool = ctx.enter_context(tc.tile_pool(name="ypool", bufs=4))
        ospool = ctx.enter_context(tc.tile_pool(name="ospool", bufs=4))

        xT = big.tile([128, NKB, T], BF16, tag="xT")
        wq = big.tile([128, NKB, NH * HD], BF16, tag="wq")
        wk = big.tile([128, NKB, HD], BF16, tag="wk")
        wv = big.tile([128, NKB, HD], BF16, tag="wv")
        wo = big.tile([128, NH, D], BF16, tag="wo")
        cos = big.tile([128, T], BF16, tag="cos")
        sin = big.tile([128, T], BF16, tag="sin")
        ltri = big.tile([128, 128], BF16, tag="ltri")
        rneg = big.tile([128, 128], BF16, tag="rneg")
        ones = big.tile([128, 128], BF16, tag="ones")
        qT = big.tile([128, NH, T], BF16, tag="qT")
        kT = big.tile([128, T], BF16, tag="kT")
        V = big.tile([128, NTB, HD], BF16, tag="V")

        # ---- loads: first xT block + wk/wv so the j-loop can start, then the
        # rest of xT streaming behind the compute, then later-needed tensors.
        nc.sync.dma_start(out=xT[:, 0, :], in_=xT_d[0:128, :])
        nc.scalar.dma_start(out=wk[:, :, :], in_=wk_d[:])
        nc.scalar.dma_start(out=wv[:, :, :], in_=wv_d[:])
        for j in range(1, NKB):
            nc.sync.dma_start(out=xT[:, j, :], in_=xT_d[j * 128 : (j + 1) * 128, :])
        nc.gpsimd.dma_start(out=ltri[:], in_=ltri_d[:])
        nc.gpsimd.dma_start(out=rneg[:], in_=rneg_d[:])
        nc.sync.dma_start(out=wq[:, :, :], in_=wq_d[:])
        nc.sync.dma_start(out=cos[:], in_=cos_d[:])
        nc.sync.dma_start(out=sin[:], in_=sin_d[:])
        nc.gpsimd.dma_start(out=wo[:, :, :], in_=wo_d[:])

        # ---- j-outer K projection (4 chunks) + V token-blocks 0..7, so PE
        # compute overlaps the xT DMA stream. 12 concurrent PSUM groups.
        kpt = [psA.tile([128, 1024], F32, tag="ps", name=f"kpt{g}") for g in range(2)]
        kps = [kpt[c // 2][:, (c % 2) * 512 : (c % 2 + 1) * 512] for c in range(NTC)]
        vtps = [
            (psV if c < 2 else psB).tile(
                [128, 512], F32, tag="v" if c < 2 else "otp", name=f"vtps{c}"
            )
            for c in range(NTC)
        ]
        for j in range(NKB):
            for c in range(NTC):
                nc.tensor.matmul(
                    kps[c],
                    lhsT=wk[:, j, :],
                    rhs=xT[:, j, c * 512 : (c + 1) * 512],
                    start=(j == 0),
                    stop=(j == NKB - 1),
                )
            for c in range(NTC):
                nc.tensor.matmul(
                    vtps[c][:],
                    lhsT=wv[:, j, :],
                    rhs=xT[:, j, c * 512 : (c + 1) * 512],
                    start=(j == 0),
                    stop=(j == NKB - 1),
                )
        nc.gpsimd.memset(ones[:], 1.0)
        VT_sb = big.tile([128, T], BF16, tag="VT_sb")
        for c in range(NTC):
            nc.scalar.copy(VT_sb[:, c * 512 : (c + 1) * 512], vtps[c][:])
        nc.scalar.dma_start_transpose(V[:, :, :], VT_sb[:])

        # ---- RoPE: dst[:, sl] = ps*cos + swap_halves(ps)*sin
        def rope(ps, c, dst_sl):
            sl = slice(c * 512, (c + 1) * 512)
            t1 = rtmp.tile([128, 512], F32, tag="t1")
            nc.vector.tensor_mul(t1[:], ps, cos[:, sl])
            t2 = rtmp.tile([128, 512], F32, tag="t2")
            nc.vector.tensor_mul(t2[0:64, :], ps[64:128, :], sin[0:64, sl])
            nc.vector.tensor_mul(t2[64:128, :], ps[0:64, :], sin[64:128, sl])
            nc.vector.tensor_add(dst_sl, t1[:], t2[:])

        for c in range(NTC):
            rope(kps[c], c, kT[:, c * 512 : (c + 1) * 512])

        # ---- Q projection + RoPE (chunk-outer so attention consumes in order)
        for c in range(NTC):
            for h in range(NH):
                n = c * NH + h
                if n >= 12:
                    # last chunk stays off the psA ring so attention's first
                    # score psums don't wait on the final rope reads
                    qp = (psV if n % 2 == 0 else psB).tile(
                        [128, 512], F32, tag="v" if n % 2 == 0 else "otp", name="qpl"
                    )[:]
                elif n % 3 == 1:
                    qp = psV.tile([128, 512], F32, tag="v", name="qpv")[:]
                elif n % 3 == 2:
                    qp = psB.tile([128, 512], F32, tag="otp", name="qpb")[:]
                else:
                    qp_t = psA.tile([128, 1024], F32, tag="ps", name="qp")
                    qp = qp_t[:, 0:512]
                for j in range(NKB):
                    nc.tensor.matmul(
                        qp[:],
                        lhsT=wq[:, j, h * 128 : (h + 1) * 128],
                        rhs=xT[:, j, c * 512 : (c + 1) * 512],
                        start=(j == 0),
                        stop=(j == NKB - 1),
                    )
                rope(qp, c, qT[:, h, c * 512 : (c + 1) * 512])

        # ---- attention + output projection, chunk-pipelined.
        # Scores are computed TRANSPOSED on the PE (ST[k,q] = kT_blk^T @ qT),
        # so exp writes P^T straight into SBUF -- no DMA/PE transposes. The
        # softmax denominator comes from an ones-weights accumulating matmul
        # over P^T (landing broadcast across partitions), the PV product is
        # OT = V^T @ P^T, both 512-wide with diagonal blocks column-trimmed.
        def S_units(h, qc, PT):
            # yields after each emitted unit so the scheduler can interleave
            # the previous head's OT/Lps matmuls between score units
            for b in range(0, 4 * qc, 2):
                spt_t = psA.tile([128, 1024], F32, tag="ps", name="spt")
                for half in range(2):
                    nc.tensor.matmul(
                        spt_t[:, half * 512 : half * 512 + 512],
                        lhsT=kT[:, (b + half) * 128 : (b + half + 1) * 128],
                        rhs=qT[:, h, qc * 512 : (qc + 1) * 512],
                        start=True,
                        stop=True,
                    )
                nc.scalar.activation(PT[:, b : b + 2, :], spt_t[:, :], EXP)
                yield
            spt_t = None
            for r in range(4):
                b = 4 * qc + r
                cl = 128 * r
                if r % 2 == 0:
                    spt_t = psA.tile([128, 1024], F32, tag="ps", name="sptd")
                reg = spt_t[:, (r % 2) * 512 : (r % 2) * 512 + 512]
                nc.tensor.matmul(
                    reg[:, cl:512],
                    lhsT=kT[:, b * 128 : (b + 1) * 128],
                    rhs=qT[:, h, qc * 512 + cl : (qc + 1) * 512],
                    start=True,
                    stop=False,
                )
                nc.tensor.matmul(
                    reg[:, cl : cl + 128],
                    lhsT=ltri[:],
                    rhs=rneg[:],
                    start=False,
                    stop=True,
                )
                nc.scalar.activation(PT[:, b, cl:512], reg[:, cl:512], EXP)
                yield

        def OT_units(h, qc, PT, ot_sb):
            nb = 4 * qc + 4
            Lp = psV.tile([128, 512], F32, tag="v", name="Lp")
            otp = psB.tile([128, 512], F32, tag="otp")
            for b in range(nb):
                r = b - 4 * qc
                cl = 128 * r if r > 0 else 0
                nc.tensor.matmul(
                    Lp[:, cl:512],
                    lhsT=ones[:],
                    rhs=PT[:, b, cl:512],
                    start=(b == 0),
                    stop=(b == nb - 1),
                )
                nc.tensor.matmul(
                    otp[:, cl:512],
                    lhsT=V[:, b, :],
                    rhs=PT[:, b, cl:512],
                    start=(b == 0),
                    stop=(b == nb - 1),
                )
                if b % 2 == 1:
                    yield
            rl_sb = ospool.tile([128, 512], F32, tag="rlb")
            nc.vector.reciprocal_approx_fast(rl_sb[:], Lp[:])
            nc.vector.tensor_mul(ot_sb[:, h, :], otp[:], rl_sb[:])
            yield

        def WO_units(qc, ot_sb):
            for i_loc in range(4):
                i = 4 * qc + i_loc
                for half in range(2):
                    ysb = ypool.tile([128, 2, 512], BF16, tag="y")
                    wp_t = psA.tile([128, 1024], F32, tag="ps", name="wp")
                    for dh in range(2):
                        dc = half * 2 + dh
                        for h in range(NH):
                            nc.tensor.matmul(
                                wp_t[:, dh * 512 : (dh + 1) * 512],
                                lhsT=ot_sb[:, h, i_loc * 128 : (i_loc + 1) * 128],
                                rhs=wo[:, h, dc * 512 : (dc + 1) * 512],
                                start=(h == 0),
                                stop=(h == NH - 1),
                            )
                    if half == 0:
                        nc.scalar.copy(ysb[:, :, :], wp_t[:])
                    else:
                        nc.vector.tensor_copy(ysb[:, :, :], wp_t[:])
                    nc.sync.dma_start(
                        out=y_d[i * 128 : (i + 1) * 128, half * 1024 : (half + 1) * 1024],
                        in_=ysb[:, :, :],
                    )
                    yield

        PTs = {}
        ot_sbs = {}
        carry = None  # deferred OT(h3, qc-1): gets an S-phase of exp slack
        for qc in range(NTC):
            ot_sbs[qc] = opool.tile([128, NH, 512], BF16, tag="ot", name=f"ot{qc}")
            for h in range(NH):
                PTs[(h, qc)] = ptpool.tile(
                    [128, NTB, 512], BF16, tag="PT", name=f"PT{h}_{qc}"
                )
                for _ in S_units(h, qc, PTs[(h, qc)]):
                    pass
                if h == 0 and carry is not None:
                    for _ in OT_units(*carry):
                        pass
                    carry = None
                if h == 2 and qc >= 1:
                    for _ in WO_units(qc - 1, ot_sbs[qc - 1]):
                        pass
                if h >= 1:
                    for _ in OT_units(h - 1, qc, PTs[(h - 1, qc)], ot_sbs[qc]):
                        pass
            carry = (NH - 1, qc, PTs[(NH - 1, qc)], ot_sbs[qc])
        for _ in OT_units(*carry):
            pass
        for _ in WO_units(NTC - 1, ot_sbs[NTC - 1]):
            pass

    nc.compile()
    _program = nc
    return nc


def _host_prep(x, Wq, Wk, Wv, Wo):
    x = np.asarray(x, dtype=np.float32)
    Wq = np.asarray(Wq, dtype=np.float32)
    Wk = np.asarray(Wk, dtype=np.float32)
    Wv = np.asarray(Wv, dtype=np.float32)
    Wo = np.asarray(Wo, dtype=np.float32)

    # RoPE even/odd gather folded into weight column permutation (per head);
    # score scale folded into Wq.
    perm128 = np.r_[np.arange(0, 128, 2), np.arange(1, 128, 2)]
    permq = np.concatenate([hb * 128 + perm128 for hb in range(H)])
    permk = np.concatenate([hb * 128 + perm128 for hb in range(KV)])
    Wq_p = Wq[:, permq] * SCALE
    Wk_p = Wk[:, permk]

    pos = np.arange(T, dtype=np.float64)
    inv_freq = 1.0 / (10000.0 ** (np.arange(0, HD, 2, dtype=np.float64) / HD))
    ang = np.einsum("t,f->tf", pos, inv_freq)  # [T, 64]
    cos = np.cos(ang).T.astype(np.float32)  # [64, T]
    sin = np.sin(ang).T.astype(np.float32)
    cosk = np.concatenate([cos, cos], axis=0).astype(bfloat16)  # [128, T]
    sink = np.concatenate([-sin, sin], axis=0).astype(bfloat16)

    ltri = np.triu(np.ones((128, 128), dtype=np.float32), k=1).astype(bfloat16)
    rneg = (np.eye(128, dtype=np.float32) * MASK_VAL).astype(bfloat16)

    def _blk(w):
        # [J*128, C] -> [128, J*C]: row-block j lands at columns [j*C,(j+1)*C)
        J = w.shape[0] // 128
        return np.ascontiguousarray(
            w.reshape(J, 128, -1).transpose(1, 0, 2).reshape(128, -1)
        ).astype(bfloat16)

    in_maps = []
    for c in range(8):
        b, s = c // 4, c % 4
        in_maps.append(
            {
                "xT": np.ascontiguousarray(x[b].T).astype(bfloat16),
                "Wq": _blk(Wq_p[:, s * 512 : (s + 1) * 512]),
                "Wk": _blk(Wk_p[:, s * 128 : (s + 1) * 128]),
                "Wv": _blk(Wv[:, s * 128 : (s + 1) * 128]),
                "Wo": _blk(Wo[s * 512 : (s + 1) * 512, :]),
                "cos": cosk,
                "sin": sink,
                "Ltri": ltri,
                "Rneg": rneg,
            }
        )
    return in_maps


def _ensure_ntff_hook():
    """The agent image's antenv lacks axon_hooks, so boot() skips installing
    the NTFF profile hook. Recreate the module and install the hook."""
    import sys
    import types

    try:
        from antenv.axon_hooks import get_axon_ntff_profile_hook  # noqa: F401

        return True
    except ImportError:
        pass
    try:
        import antenv
        from trn_agent_boot.trn_boot import _ntff_profile_via_ctypes

        hook = _ntff_profile_via_ctypes("/opt/axon/libaxon_pjrt.so")
        if hook is None:
            return False
        mod = types.ModuleType("antenv.axon_hooks")
        mod._hook = hook
        mod.set_axon_ntff_profile_hook = lambda h: setattr(mod, "_hook", h)
        mod.get_axon_ntff_profile_hook = lambda: mod._hook
        sys.modules["antenv.axon_hooks"] = mod
        antenv.axon_hooks = mod
        bass_utils.upload_artifacts = lambda d: d
        return True
    except Exception:
        return False


def kernel(x, Wq, Wk, Wv, Wo):
    global _last_results, last_exec_time_ns
    nc = _build_program()
    in_maps = _host_prep(x, Wq, Wk, Wv, Wo)
    trace = bool(int(os.environ.get("KERNEL_TRACE", "0")))
    tmpdir = None
    if trace:
        trace = _ensure_ntff_hook()
        if trace:
            tmpdir = os.environ.get("KERNEL_TRACE_DIR") or None
    res = bass_utils.run_bass_kernel_spmd(
        nc, in_maps, core_ids=list(range(8)), trace=trace, tmpdir=tmpdir
    )
    _last_results = res
    last_exec_time_ns = res.exec_time_ns
    out = np.empty((B, T, D), dtype=np.float32)
    for b in range(B):
        out[b] = sum(
            res.results[4 * b + s]["y"].astype(np.float32) for s in range(TP)
        )
    return out
